# revision 11
# baseline (speedup 1.0000x reference)
"""MoE FFN (shared SwiGLU + 8 dense-routed SwiGLU experts) on 8 TRN2 NeuronCores.

Strategy: data-parallel over batch (B=16 -> 2 batches/core), expert weights
replicated. All activations kept feature-major ([feature, token]) so every
matmul consumes natural-layout weights and host-pre-transposed x with zero
on-chip transposes. Matmul operands are bf16 (FWL weight loads hide under the
N=512 moving stream); accumulation is fp32 in PSUM. The routed mixture weight
and all down-projection biases are folded in for free (rw into the up-branch
elementwise op, biases into the first unit's PSUM->SBUF accumulate).
"""
import sys

if "/opt/trn_rl_repo" not in sys.path:
    sys.path.insert(0, "/opt/trn_rl_repo")

import numpy as np
import ml_dtypes

import concourse.bass as bass  # noqa: F401  (registers engine classes)
import concourse.tile as tile
from concourse import bacc, mybir
from concourse import bass_utils

F32 = mybir.dt.float32
BF16 = mybir.dt.bfloat16
Silu = mybir.ActivationFunctionType.Silu
Alu = mybir.AluOpType

B, K, D = 16, 1024, 512
HS, HR, E = 2048, 1024, 8
NCORES = 8
BL = B // NCORES          # batches per core = 2
T = BL * K                # tokens per core = 2048
TT = 512                  # token tile (matmul moving dim)
NTT = T // TT             # 4 token tiles per core
NU = 2 + E                # units: 2 shared halves + 8 experts
HU = 1024                 # hidden width of every unit
NH = HU // 128            # 8 h-tiles per unit
ND = D // 128             # 4 d-tiles
NK = D // 128             # 4 contraction tiles for gate/up


def _build():
    nc = bacc.Bacc("TRN2", target_bir_lowering=False, debug=False,
                   num_devices=NCORES)
    xT = nc.dram_tensor("xT", (D, T), BF16, kind="ExternalInput")
    wg = nc.dram_tensor("wg", (NU, D, HU), BF16, kind="ExternalInput")
    wu = nc.dram_tensor("wu", (NU, D, HU), BF16, kind="ExternalInput")
    wd = nc.dram_tensor("wd", (NU, HU, D), BF16, kind="ExternalInput")
    gb = nc.dram_tensor("gb", (128, NU, NH), F32, kind="ExternalInput")
    ub = nc.dram_tensor("ub", (128, NU, NH), F32, kind="ExternalInput")
    rw = nc.dram_tensor("rw", (128, NU, NTT), F32, kind="ExternalInput")
    cv = nc.dram_tensor("cv", (128, ND, NTT), F32, kind="ExternalInput")
    outT = nc.dram_tensor("outT", (D, T), F32, kind="ExternalOutput")

    with tile.TileContext(nc) as tc:
        with (
            tc.tile_pool(name="persist", bufs=1) as persist,
            tc.tile_pool(name="wpool", bufs=2) as wpool,
            tc.tile_pool(name="hpool", bufs=2) as hpool,
            tc.tile_pool(name="spool", bufs=4) as spool,
            tc.tile_pool(name="gups", bufs=2, space="PSUM") as gups,
            tc.tile_pool(name="ops", bufs=1, space="PSUM") as opsp,
        ):
            xt = persist.tile([128, NK, T], BF16)
            oacc = persist.tile([128, ND, T], F32)
            gbt = persist.tile([128, NU, NH], F32)
            ubt = persist.tile([128, NU, NH], F32)
            rwt = persist.tile([128, NU, NTT], F32)
            cvt = persist.tile([128, ND, NTT], F32)

            def load_unit_weights(u, wgt, wut, wdt, first=False):
                # split gate/up weight loads by h-half so the first matmuls of
                # a unit only wait on the slices they actually read; for the
                # first unit, interleave with the x loads the same matmuls
                # need so the critical descriptors spread across DMA queues.
                for half in range(2):
                    hh = slice(half * 512, (half + 1) * 512)
                    for k in range(NK):
                        nc.sync.dma_start(wgt[:, k, hh],
                                          wg.ap()[u, k * 128:(k + 1) * 128, hh])
                        if first:
                            nc.sync.dma_start(
                                xt[:, k, slice(half * TT, (half + 1) * TT)],
                                xT.ap()[k * 128:(k + 1) * 128,
                                        half * TT:(half + 1) * TT])
                    if first and half == 0:
                        nc.sync.dma_start(gbt[:], gb.ap()[:])
                        nc.sync.dma_start(ubt[:], ub.ap()[:])
                        nc.sync.dma_start(rwt[:], rw.ap()[:])
                        nc.sync.dma_start(cvt[:], cv.ap()[:])
                    for k in range(NK):
                        nc.sync.dma_start(wut[:, k, hh],
                                          wu.ap()[u, k * 128:(k + 1) * 128, hh])
                for k in range(NH):
                    nc.sync.dma_start(wdt[:, k, :], wd.ap()[u, k * 128:(k + 1) * 128, :])

            w0 = (wpool.tile([128, NK, HU], BF16, tag="wg", name="wgt0"),
                  wpool.tile([128, NK, HU], BF16, tag="wu", name="wut0"),
                  wpool.tile([128, NH, D], BF16, tag="wd", name="wdt0"))
            load_unit_weights(0, *w0, first=True)
            # remaining x token tiles
            for t in range(2, NTT):
                tok = slice(t * TT, (t + 1) * TT)
                for k in range(NK):
                    nc.sync.dma_start(xt[:, k, tok],
                                      xT.ap()[k * 128:(k + 1) * 128, tok])

            for u in range(NU):
                if u == 0:
                    wgt, wut, wdt = w0
                else:
                    wgt = wpool.tile([128, NK, HU], BF16, tag="wg")
                    wut = wpool.tile([128, NK, HU], BF16, tag="wu")
                    wdt = wpool.tile([128, NH, D], BF16, tag="wd")
                    load_unit_weights(u, wgt, wut, wdt)

                for t in range(NTT):
                    tok = slice(t * TT, (t + 1) * TT)
                    hts = [hpool.tile([128, TT], BF16, tag=f"h{hi}",
                                      name=f"h{hi}_u{u}t{t}") for hi in range(NH)]
                    for hi in range(NH):
                        hc = slice(hi * 128, (hi + 1) * 128)
                        gps = gups.tile([128, TT], F32, tag="g")
                        for k in range(NK):
                            nc.tensor.matmul(gps[:], wgt[:, k, hc], xt[:, k, tok],
                                             start=(k == 0), stop=(k == NK - 1))
                        ups = gups.tile([128, TT], F32, tag="u")
                        for k in range(NK):
                            nc.tensor.matmul(ups[:], wut[:, k, hc], xt[:, k, tok],
                                             start=(k == 0), stop=(k == NK - 1))
                        sg = spool.tile([128, TT], F32, tag="sg")
                        nc.scalar.activation(sg[:], gps[:], Silu,
                                             bias=gbt[:, u, hi:hi + 1])
                        su = spool.tile([128, TT], F32, tag="su")
                        nc.vector.tensor_scalar(su[:], ups[:],
                                                ubt[:, u, hi:hi + 1],
                                                rwt[:, u, t:t + 1],
                                                Alu.add, Alu.mult)
                        nc.vector.tensor_tensor(hts[hi][:], sg[:], su[:], Alu.mult)

                    odst = [opsp.tile([128, TT], F32, tag=f"o{di}",
                                      name=f"o{di}_u{u}t{t}") for di in range(ND)]
                    for k in range(NH):
                        for di in range(ND):
                            nc.tensor.matmul(odst[di][:],
                                             wdt[:, k, di * 128:(di + 1) * 128],
                                             hts[k][:],
                                             start=(k == 0), stop=(k == NH - 1),
                                             skip_group_check=True)
                    for di in range(ND):
                        dst = oacc[:, di, tok]
                        if u == 0:
                            nc.vector.tensor_scalar_add(dst, odst[di][:],
                                                        cvt[:, di, t:t + 1])
                        else:
                            nc.vector.tensor_tensor(dst, odst[di][:], dst, Alu.add)
                        if u == NU - 1:
                            nc.sync.dma_start(
                                outT.ap()[di * 128:(di + 1) * 128, tok], dst)
    nc.compile()
    return nc


_NC = None


def _get_nc():
    global _NC
    if _NC is None:
        _NC = _build()
    return _NC


def _bf16(a):
    return np.ascontiguousarray(np.asarray(a, np.float32)).astype(ml_dtypes.bfloat16)


def _pack_shared(Ws_gate, bs_gate, Ws_up, bs_up, Ws_down, bs_down,
                 Wr_gate, br_gate, Wr_up, br_up, Wr_down, br_down):
    wg = np.empty((NU, D, HU), np.float32)
    wu = np.empty((NU, D, HU), np.float32)
    wd = np.empty((NU, HU, D), np.float32)
    gb = np.empty((128, NU, NH), np.float32)
    ub = np.empty((128, NU, NH), np.float32)
    for u in range(2):
        h0 = slice(u * HU, (u + 1) * HU)
        wg[u] = Ws_gate[:, h0]
        wu[u] = Ws_up[:, h0]
        wd[u] = Ws_down[h0, :]
        gb[:, u, :] = np.asarray(bs_gate[h0]).reshape(NH, 128).T
        ub[:, u, :] = np.asarray(bs_up[h0]).reshape(NH, 128).T
    for e in range(E):
        wg[2 + e] = Wr_gate[e]
        wu[2 + e] = Wr_up[e]
        wd[2 + e] = Wr_down[e]
        gb[:, 2 + e, :] = np.asarray(br_gate[e]).reshape(NH, 128).T
        ub[:, 2 + e, :] = np.asarray(br_up[e]).reshape(NH, 128).T
    return (_bf16(wg), _bf16(wu), _bf16(wd),
            np.ascontiguousarray(gb), np.ascontiguousarray(ub))


def _run(inputs, trace=False):
    x = np.asarray(inputs["x"], np.float32)
    rweights = np.asarray(inputs["routing_weights"], np.float32)
    wg, wu, wd, gb, ub = _pack_shared(
        np.asarray(inputs["Ws_gate"], np.float32), inputs["bs_gate"],
        np.asarray(inputs["Ws_up"], np.float32), inputs["bs_up"],
        np.asarray(inputs["Ws_down"], np.float32), inputs["bs_down"],
        np.asarray(inputs["Wr_gate"], np.float32), inputs["br_gate"],
        np.asarray(inputs["Wr_up"], np.float32), inputs["br_up"],
        np.asarray(inputs["Wr_down"], np.float32), inputs["br_down"])
    bs_down = np.asarray(inputs["bs_down"], np.float32)
    br_down = np.asarray(inputs["br_down"], np.float32)
    # down-bias vector per batch: bs_down + sum_e rw[b,e]*br_down[e]
    cfull = bs_down[None, :] + rweights @ br_down       # [B, D]

    in_maps = []
    for i in range(NCORES):
        xT = _bf16(x[BL * i:BL * (i + 1)].reshape(T, D).T)
        rw = np.ones((128, NU, NTT), np.float32)
        cv = np.empty((128, ND, NTT), np.float32)
        for t in range(NTT):
            bg = BL * i + t // (K // TT)
            for e in range(E):
                rw[:, 2 + e, t] = rweights[bg, e]
            cv[:, :, t] = cfull[bg].reshape(ND, 128).T
        in_maps.append({"xT": np.ascontiguousarray(xT), "wg": wg, "wu": wu,
                        "wd": wd, "gb": gb, "ub": ub,
                        "rw": np.ascontiguousarray(rw),
                        "cv": np.ascontiguousarray(cv)})

    res = bass_utils.run_bass_kernel_spmd(_get_nc(), in_maps,
                                          core_ids=list(range(NCORES)),
                                          trace=trace)
    out = np.empty((B, K, D), np.float32)
    for i in range(NCORES):
        out[BL * i:BL * (i + 1)] = res.results[i]["outT"].T.reshape(BL, K, D)
    return out, res


def kernel(**inputs) -> np.ndarray:
    out, _ = _run(inputs, trace=False)
    return out
